# revision 1
# baseline (speedup 1.0000x reference)
"""DeepSeek-V3-style MoE kernel for Trainium2, 8-core expert-parallel.

Sharding:
  - Routed experts (E=64) sharded 8 per core (expert parallel). Core k owns
    experts [8k, 8k+8). Host permutes WHOLE GROUPS (group size == 8 == EL) so
    each core's local experts occupy score columns [0, 8); group-limited
    routing is invariant under whole-group permutation.
  - Shared expert tensor-parallel over IS (2048 -> 256 per core).
  - Gate + hidden replicated; host sums the 8 partial outputs (all-reduce).

On-device per core:
  phase A (per 128-token tile): DMA hidden tile, PE-transpose to [h, t],
    router GEMM, sigmoid+bias, group top-2 sums (DVE max8), top-4 groups and
    top-8 experts via per-row thresholds, combine weights; shared-expert
    gated MLP for the tile. Local-expert cw columns staged to cw_loc.
  phase B: rewrap cw_loc into per-expert [16, 256] lanes, build (token-id,
    cw) arrays with -1 holes, gpsimd sparse_gather compacts each expert's
    token list to capacity CAP.
  phase D (per local expert): dma_gather token rows, PE-transpose, gated MLP
    GEMMs (f32), scale by cw, dma_scatter_add into the partial output.
"""

import sys
import numpy as np

sys.path.insert(0, "/opt/trn_rl_repo")

T, H, E, I, IS = 4096, 1024, 64, 512, 2048
N_GROUP, TOPK_GROUP, TOP_K = 8, 4, 8
ROUTED_SCALE = 2.5

NCORES = 8
EL = E // NCORES
ISL = IS // NCORES
TT = T // 128
HB = H // 128
IB = I // 128
ISB = ISL // 128


DEBUG_DUMPS = False


def build_kernel(caps):
    # caps: per-local-slot token capacities (multiples of 128)
    from concourse import bacc, mybir, tile

    f32 = mybir.dt.float32
    i16 = mybir.dt.int16
    u32 = mybir.dt.uint32
    AF = mybir.ActivationFunctionType
    OP = mybir.AluOpType
    AX = mybir.AxisListType

    nc = bacc.Bacc("TRN2", target_bir_lowering=False, debug=False,
                   num_devices=NCORES)

    hid = nc.declare_dram_parameter("hid", [T, H], f32, isOutput=False)
    gwt = nc.declare_dram_parameter("gwt", [H, E], f32, isOutput=False)
    ebias = nc.declare_dram_parameter("ebias", [128, E], f32, isOutput=False)
    tok1 = nc.declare_dram_parameter("tok1", [16, EL, T // 16], f32,
                                     isOutput=False)
    ident_in = nc.declare_dram_parameter("ident", [128, 128], f32,
                                         isOutput=False)
    nposc_in = nc.declare_dram_parameter("nposc", [128, 64], f32,
                                         isOutput=False)
    nposi_in = nc.declare_dram_parameter("nposi", [16, 512], f32,
                                         isOutput=False)
    w1t = nc.declare_dram_parameter("w1t", [EL, H, I], f32, isOutput=False)
    w3t = nc.declare_dram_parameter("w3t", [EL, H, I], f32, isOutput=False)
    w2t = nc.declare_dram_parameter("w2t", [EL, I, H], f32, isOutput=False)
    ws1t = nc.declare_dram_parameter("ws1t", [H, ISL], f32, isOutput=False)
    ws3t = nc.declare_dram_parameter("ws3t", [H, ISL], f32, isOutput=False)
    ws2t = nc.declare_dram_parameter("ws2t", [ISL, H], f32, isOutput=False)
    out_d = nc.declare_dram_parameter("out", [T, H], f32, isOutput=True)
    if DEBUG_DUMPS:
        capm = max(caps)
        dbg_cwl = nc.declare_dram_parameter("dbg_cwl", [128, TT, EL], f32,
                                            isOutput=True)
        dbg_wv = nc.declare_dram_parameter("dbg_wv", [16, EL, T // 16], f32,
                                           isOutput=True)
        dbg_cnt = nc.declare_dram_parameter("dbg_cnt", [EL, 2], u32,
                                            isOutput=True)
        dbg_idx = nc.declare_dram_parameter("dbg_idx", [EL, 128, capm // 16],
                                            i16, isOutput=True)
        dbg_cwv = nc.declare_dram_parameter("dbg_cwv", [EL, 128, capm // 128],
                                            f32, isOutput=True)
        dbg_xg0 = nc.declare_dram_parameter("dbg_xg0", [128, caps[0] // 128, H],
                                            f32, isOutput=True)
        dbg_y0 = nc.declare_dram_parameter("dbg_y0", [128, caps[0] // 128, H],
                                           f32, isOutput=True)

    import contextlib
    with tile.TileContext(nc) as tc, contextlib.ExitStack() as ctx:
        p_const = ctx.enter_context(tc.tile_pool(name="const", bufs=1))
        p_disp = ctx.enter_context(tc.tile_pool(name="disp", bufs=1))
        ps_a = ctx.enter_context(tc.tile_pool(name="ps_a", bufs=2,
                                              space="PSUM"))
        ps_b = ctx.enter_context(tc.tile_pool(name="ps_b", bufs=1,
                                              space="PSUM"))
        ps_y = ctx.enter_context(tc.tile_pool(name="ps_y", bufs=1,
                                              space="PSUM"))

        g_sem = nc.alloc_semaphore("gather_done")
        sc_sem = nc.alloc_semaphore("scatter_done")
        ident = p_const.tile([128, 128], f32, tag="ident")
        nc.sync.dma_start(out=ident[:], in_=ident_in[:])
        nposc = p_const.tile([128, 64], f32, tag="nposc")
        nc.sync.dma_start(out=nposc[:], in_=nposc_in[:])
        nposi = p_const.tile([16, 512], f32, tag="nposi")
        nc.sync.dma_start(out=nposi[:], in_=nposi_in[:])
        ones_row = p_const.tile([1, 128], f32, tag="ones_row")
        nc.vector.memset(ones_row[:], 1.0)
        eb_sb = p_const.tile([128, E], f32, tag="eb")
        nc.sync.dma_start(out=eb_sb[:], in_=ebias[:])
        gwt_sb = p_const.tile([128, HB, E], f32, tag="gwt")
        for hb in range(HB):
            nc.sync.dma_start(out=gwt_sb[:, hb, :],
                              in_=gwt[hb * 128:(hb + 1) * 128, :])
        cw_loc = p_const.tile([128, TT, EL], f32, tag="cw_loc")

        # ---------------- phase A ----------------
        with tc.tile_pool(name="hin", bufs=2) as p_in, \
             tc.tile_pool(name="hT", bufs=3) as p_hT, \
             tc.tile_pool(name="rt", bufs=2) as p_rt, \
             tc.tile_pool(name="sw", bufs=1) as p_sw, \
             tc.tile_pool(name="sact", bufs=2) as p_sact:

            ws1t_sb = p_sw.tile([128, HB, ISL], f32, tag="ws1")
            ws3t_sb = p_sw.tile([128, HB, ISL], f32, tag="ws3")
            for hb in range(HB):
                nc.sync.dma_start(out=ws1t_sb[:, hb, :],
                                  in_=ws1t[hb * 128:(hb + 1) * 128, :])
                nc.sync.dma_start(out=ws3t_sb[:, hb, :],
                                  in_=ws3t[hb * 128:(hb + 1) * 128, :])
            ws2t_sb = p_sw.tile([128, ISB, H], f32, tag="ws2")
            for ib in range(ISB):
                nc.sync.dma_start(out=ws2t_sb[:, ib, :],
                                  in_=ws2t[ib * 128:(ib + 1) * 128, :])

            for tt in range(TT):
                ts = slice(tt * 128, (tt + 1) * 128)
                htile = p_in.tile([128, H], f32, tag="hin")
                nc.sync.dma_start(out=htile[:], in_=hid[ts, :])
                hTt = p_hT.tile([128, HB, 128], f32, tag="hTt")
                for g in range(2):
                    tp = ps_a.tile([128, 4, 128], f32, tag="tp")
                    for j in range(4):
                        hb = g * 4 + j
                        nc.tensor.transpose(
                            out=tp[:, j, :],
                            in_=htile[:, hb * 128:(hb + 1) * 128],
                            identity=ident[:])
                    nc.any.tensor_copy(out=hTt[:, g * 4:(g + 1) * 4, :],
                                       in_=tp[:])

                lg = ps_a.tile([128, E], f32, tag="lg")
                for hb in range(HB):
                    nc.tensor.matmul(out=lg[:], lhsT=hTt[:, hb, :],
                                     rhs=gwt_sb[:, hb, :],
                                     start=(hb == 0), stop=(hb == HB - 1))

                scores = p_rt.tile([128, E], f32, tag="scores")
                nc.scalar.activation(scores[:], lg[:], AF.Sigmoid)
                swb = p_rt.tile([128, E], f32, tag="swb")
                nc.vector.tensor_add(swb[:], scores[:], eb_sb[:])

                grp = p_rt.tile([128, N_GROUP], f32, tag="grp")
                mx8 = p_rt.tile([128, 8], f32, tag="mx8")
                for g in range(N_GROUP):
                    nc.vector.max(out=mx8[:], in_=swb[:, g * 8:(g + 1) * 8])
                    nc.vector.tensor_add(grp[:, g:g + 1], mx8[:, 0:1],
                                         mx8[:, 1:2])
                gm8 = p_rt.tile([128, 8], f32, tag="gm8")
                nc.vector.max(out=gm8[:], in_=grp[:])
                gmask = p_rt.tile([128, N_GROUP], f32, tag="gmask")
                nc.vector.tensor_scalar(
                    out=gmask[:], in0=grp[:],
                    scalar1=gm8[:, TOPK_GROUP - 1:TOPK_GROUP],
                    scalar2=None, op0=OP.is_ge)
                mswb = p_rt.tile([128, E], f32, tag="mswb")
                nc.vector.tensor_tensor(
                    out=mswb[:], in0=swb[:],
                    in1=gmask[:].to_broadcast([128, N_GROUP, 8]), op=OP.mult)
                tm8 = p_rt.tile([128, 8], f32, tag="tm8")
                nc.vector.max(out=tm8[:], in_=mswb[:])
                nmask = p_rt.tile([128, E], f32, tag="nmask")
                nc.vector.tensor_scalar(
                    out=nmask[:], in0=mswb[:],
                    scalar1=tm8[:, TOP_K - 1:TOP_K],
                    scalar2=None, op0=OP.is_ge)
                s_sel = p_rt.tile([128, E], f32, tag="s_sel")
                nc.vector.tensor_tensor(out=s_sel[:], in0=scores[:],
                                        in1=nmask[:], op=OP.mult)
                rsum = p_rt.tile([128, 1], f32, tag="rsum")
                nc.vector.tensor_reduce(out=rsum[:], in_=s_sel[:], axis=AX.X,
                                        op=OP.add)
                rinv = p_rt.tile([128, 1], f32, tag="rinv")
                nc.vector.reciprocal(rinv[:], rsum[:])
                nc.vector.tensor_scalar_mul(rinv[:], rinv[:], ROUTED_SCALE)
                nc.vector.tensor_scalar_mul(cw_loc[:, tt, :], s_sel[:, 0:EL],
                                            rinv[:])

                # shared expert for this tile
                h1s = ps_b.tile([128, ISL], f32, tag="h1")
                h3s = ps_b.tile([128, ISL], f32, tag="h3")
                for hb in range(HB):
                    nc.tensor.matmul(out=h1s[:], lhsT=hTt[:, hb, :],
                                     rhs=ws1t_sb[:, hb, :],
                                     start=(hb == 0), stop=(hb == HB - 1))
                for hb in range(HB):
                    nc.tensor.matmul(out=h3s[:], lhsT=hTt[:, hb, :],
                                     rhs=ws3t_sb[:, hb, :],
                                     start=(hb == 0), stop=(hb == HB - 1))
                sil = p_sact.tile([128, ISL], f32, tag="sil")
                nc.scalar.activation(sil[:], h1s[:], AF.Silu)
                acts = p_sact.tile([128, ISL], f32, tag="acts")
                nc.vector.tensor_tensor(out=acts[:], in0=sil[:], in1=h3s[:],
                                        op=OP.mult)
                actsT = p_sact.tile([128, ISB, 128], f32, tag="actsT")
                tps = ps_a.tile([128, ISB, 128], f32, tag="tp")
                for ib in range(ISB):
                    nc.tensor.transpose(out=tps[:, ib, :],
                                        in_=acts[:, ib * 128:(ib + 1) * 128],
                                        identity=ident[:])
                nc.any.tensor_copy(out=actsT[:], in_=tps[:])
                ys = ps_y.tile([128, H], f32, tag="y")
                for nh in range(2):
                    for ib in range(ISB):
                        nc.tensor.matmul(
                            out=ys[:, nh * 512:(nh + 1) * 512],
                            lhsT=actsT[:, ib, :],
                            rhs=ws2t_sb[:, ib, nh * 512:(nh + 1) * 512],
                            start=(ib == 0), stop=(ib == ISB - 1))
                ysb = p_sact.tile([128, H], f32, tag="ysb")
                nc.any.tensor_copy(out=ysb[:], in_=ys[:])
                nc.sync.dma_start(out=out_d[ts, :], in_=ysb[:])

        # ---------------- phase B: dispatch ----------------
        idx_reps = []
        idx_repgs = []
        cwv_reps = []
        cnt_tiles = []
        with tc.tile_pool(name="wrap", bufs=1) as p_wr:
            wv_all = p_wr.tile([16, EL, T // 16], f32, tag="wv")
            for e in range(EL):
                nc.sync.dma_start(out=wv_all[:, e, :], in_=cw_loc[:, :, e])
            if DEBUG_DUMPS:
                nc.sync.dma_start(out=dbg_cwl[:], in_=cw_loc[:])
                nc.sync.dma_start(out=dbg_wv[:], in_=wv_all[:])
            tok1_sb = p_wr.tile([16, EL, T // 16], f32, tag="tok1")
            nc.sync.dma_start(out=tok1_sb[:], in_=tok1[:])
            sel = p_wr.tile([16, EL, T // 16], f32, tag="sel")
            nc.vector.tensor_scalar(out=sel[:], in0=wv_all[:], scalar1=0.0,
                                    scalar2=None, op0=OP.is_gt)
            wi_all = p_wr.tile([16, EL, T // 16], f32, tag="wi")
            nc.vector.tensor_tensor(out=wi_all[:], in0=tok1_sb[:],
                                    in1=sel[:], op=OP.mult)
            nc.vector.tensor_scalar_add(wi_all[:], wi_all[:], -1.0)
            nc.vector.tensor_add(wv_all[:], wv_all[:], sel[:])
            nc.vector.tensor_scalar_add(wv_all[:], wv_all[:], -1.0)

            for e in range(EL):
                CAPe = caps[e]
                CBe = CAPe // 128
                wi_o = p_disp.tile([16, CAPe // 16], f32, tag=f"wi_o{e}")
                cnt = p_disp.tile([1, 1], u32, tag=f"cnt{e}")
                nc.vector.memset(wi_o[:], -1.0)
                nc.gpsimd.sparse_gather(out=wi_o[:], in_=wi_all[:, e, :],
                                        num_found=cnt[:])
                wv_o2 = p_disp.tile([16, CAPe // 16], f32, tag=f"wv_o2{e}")
                cnt2 = p_disp.tile([1, 1], u32, tag=f"cnt2{e}")
                nc.vector.memset(wv_o2[:], -1.0)
                nc.gpsimd.sparse_gather(out=wv_o2[:], in_=wv_all[:, e, :],
                                        num_found=cnt2[:])
                wv_o = p_disp.tile([16, CBe, 8], f32, tag=f"wv_o{e}")
                nc.vector.tensor_copy(wv_o[:], wv_o2[:])

                # broadcast count across partitions
                cnt_f = p_disp.tile([1, 1], f32, tag=f"cntf{e}")
                nc.vector.tensor_copy(cnt_f[:], cnt[:])
                nbc_ps = ps_a.tile([128, 1], f32, tag="lg")
                nc.tensor.matmul(out=nbc_ps[:], lhsT=ones_row[:],
                                 rhs=cnt_f[:], start=True, stop=True)
                nbc = p_disp.tile([128, 1], f32, tag=f"nbc{e}")
                nc.vector.tensor_copy(nbc[:], nbc_ps[:])

                # idx: tail (pos >= count) := -1 so the DGE skips those rows
                keep_i = p_disp.tile([16, CAPe // 16], u32, tag=f"keepi{e}")
                nc.vector.tensor_scalar(out=keep_i[:],
                                        in0=nposi[:, :CAPe // 16],
                                        scalar1=nbc[0:16, :], scalar2=None,
                                        op0=OP.add)
                nc.vector.tensor_scalar(out=keep_i[:], in0=keep_i[:],
                                        scalar1=0.0, scalar2=None,
                                        op0=OP.is_gt)
                wi_sel = p_disp.tile([16, CAPe // 16], f32, tag=f"wisel{e}")
                nc.vector.memset(wi_sel[:], -1.0)
                nc.vector.copy_predicated(wi_sel[:], keep_i[:], wi_o[:])
                wi_i16 = p_disp.tile([16, CAPe // 16], i16, tag=f"wi16{e}")
                nc.vector.tensor_copy(wi_i16[:], wi_sel[:])
                idx_rep = p_disp.tile([128, CAPe // 16], i16, tag=f"irep{e}")
                for pg in range(8):
                    nc.sync.dma_start(out=idx_rep[pg * 16:(pg + 1) * 16, :],
                                      in_=wi_i16[:])
                # gather variant: tails clamped to 0 (static count reads them)
                nc.vector.tensor_scalar_max(wi_sel[:], wi_sel[:], 0.0)
                wi_i16g = p_disp.tile([16, CAPe // 16], i16, tag=f"wi16g{e}")
                nc.vector.tensor_copy(wi_i16g[:], wi_sel[:])
                idx_repg = p_disp.tile([128, CAPe // 16], i16, tag=f"irepg{e}")
                for pg in range(8):
                    nc.sync.dma_start(out=idx_repg[pg * 16:(pg + 1) * 16, :],
                                      in_=wi_i16g[:])

                # cw values: relu then zero the tail
                cwv = p_disp.tile([128, CBe], f32, tag=f"cwv{e}")
                for pg in range(8):
                    nc.sync.dma_start(out=cwv[pg * 16:(pg + 1) * 16, :],
                                      in_=wv_o[:, :, pg])
                nc.vector.tensor_scalar_max(cwv[:], cwv[:], 0.0)
                keep = p_disp.tile([128, CBe], f32, tag=f"keep{e}")
                nc.vector.tensor_scalar(out=keep[:], in0=nposc[:, :CBe],
                                        scalar1=nbc[:], scalar2=None,
                                        op0=OP.add)
                nc.vector.tensor_scalar(out=keep[:], in0=keep[:], scalar1=0.0,
                                        scalar2=None, op0=OP.is_gt)
                nc.vector.tensor_tensor(out=cwv[:], in0=cwv[:], in1=keep[:],
                                        op=OP.mult)
                if DEBUG_DUMPS:
                    nc.sync.dma_start(out=dbg_cnt[e, 0:1], in_=cnt[:])
                    nc.sync.dma_start(out=dbg_cnt[e, 1:2], in_=cnt2[:])
                    nc.sync.dma_start(out=dbg_idx[e, :, :CAPe // 16],
                                      in_=idx_rep[:])
                    nc.sync.dma_start(out=dbg_cwv[e, :, :CBe], in_=cwv[:])
                idx_reps.append(idx_rep)
                idx_repgs.append(idx_repg)
                cwv_reps.append(cwv)
                cnt_tiles.append(cnt)

        # ---------------- phase D: routed experts ----------------
        with tc.tile_pool(name="w13", bufs=2) as p_w13, \
             tc.tile_pool(name="w2", bufs=1) as p_w2, \
             tc.tile_pool(name="xg", bufs=1) as p_xg, \
             tc.tile_pool(name="xg2", bufs=2) as p_xg2, \
             tc.tile_pool(name="sm", bufs=2) as p_sm, \
             tc.tile_pool(name="y", bufs=1) as p_y:
            for e in range(EL):
                CAPe = caps[e]
                CBe = CAPe // 128
                w1sb = p_w13.tile([128, HB, I], f32, tag="w1")
                w3sb = p_w13.tile([128, HB, I], f32, tag="w3")
                for hb in range(HB):
                    nc.sync.dma_start(out=w1sb[:, hb, :],
                                      in_=w1t[e, hb * 128:(hb + 1) * 128, :])
                    nc.sync.dma_start(out=w3sb[:, hb, :],
                                      in_=w3t[e, hb * 128:(hb + 1) * 128, :])
                w2sb = p_w2.tile([128, IB, H], f32, tag="w2")
                for ib in range(IB):
                    nc.sync.dma_start(out=w2sb[:, ib, :],
                                      in_=w2t[e, ib * 128:(ib + 1) * 128, :])

                xg = p_xg.tile([128, CBe, H], f32, tag="xg")
                nc.gpsimd.dma_gather(
                    out_ap=xg[:], in_ap=hid[:], idxs_ap=idx_repgs[e][:],
                    num_idxs=CAPe, num_idxs_reg=CAPe, elem_size=H)

                if DEBUG_DUMPS and e == 0:
                    nc.sync.dma_start(out=dbg_xg0[:], in_=xg[:])
                xgT = p_xg2.tile([128, HB, CAPe], f32, tag="xgT")
                for b in range(CBe):
                    for g in range(2):
                        tp = ps_a.tile([128, 4, 128], f32, tag="tp")
                        for j in range(4):
                            hb = g * 4 + j
                            nc.tensor.transpose(
                                out=tp[:, j, :],
                                in_=xg[:, b, hb * 128:(hb + 1) * 128],
                                identity=ident[:])
                        nc.any.tensor_copy(
                            out=xgT[:, g * 4:(g + 1) * 4,
                                    b * 128:(b + 1) * 128],
                            in_=tp[:])

                y_sb = p_y.tile([128, CBe, H], f32, tag="ysb")
                for b in range(CBe):
                    h1 = ps_b.tile([128, I], f32, tag="h1")
                    h3 = ps_b.tile([128, I], f32, tag="h3")
                    for hb in range(HB):
                        nc.tensor.matmul(
                            out=h1[:], lhsT=xgT[:, hb, b * 128:(b + 1) * 128],
                            rhs=w1sb[:, hb, :],
                            start=(hb == 0), stop=(hb == HB - 1))
                    for hb in range(HB):
                        nc.tensor.matmul(
                            out=h3[:], lhsT=xgT[:, hb, b * 128:(b + 1) * 128],
                            rhs=w3sb[:, hb, :],
                            start=(hb == 0), stop=(hb == HB - 1))
                    sil = p_sm.tile([128, I], f32, tag="sil_r")
                    nc.scalar.activation(sil[:], h1[:], AF.Silu)
                    act = p_sm.tile([128, I], f32, tag="act_r")
                    nc.vector.tensor_tensor(out=act[:], in0=sil[:], in1=h3[:],
                                            op=OP.mult)
                    nc.vector.tensor_scalar_mul(act[:], act[:],
                                                cwv_reps[e][:, b:b + 1])
                    actT = p_sm.tile([128, IB, 128], f32, tag="actT")
                    tpa = ps_a.tile([128, IB, 128], f32, tag="tp")
                    for ib in range(IB):
                        nc.tensor.transpose(
                            out=tpa[:, ib, :],
                            in_=act[:, ib * 128:(ib + 1) * 128],
                            identity=ident[:])
                    nc.any.tensor_copy(out=actT[:], in_=tpa[:])
                    yps = ps_y.tile([128, H], f32, tag="y")
                    for nh in range(2):
                        for ib in range(IB):
                            nc.tensor.matmul(
                                out=yps[:, nh * 512:(nh + 1) * 512],
                                lhsT=actT[:, ib, :],
                                rhs=w2sb[:, ib, nh * 512:(nh + 1) * 512],
                                start=(ib == 0), stop=(ib == IB - 1))
                    nc.any.tensor_copy(out=y_sb[:, b, :], in_=yps[:])

                if DEBUG_DUMPS and e == 0:
                    nc.sync.dma_start(out=dbg_y0[:], in_=y_sb[:])
                with tc.tile_critical():
                    creg2 = nc.gpsimd.alloc_register()
                    nc.gpsimd.reg_load(creg2, cnt_tiles[e][:])
                    if e > 0:
                        nc.gpsimd.wait_ge(sc_sem, 16 * e)
                    nc.gpsimd.dma_scatter_add(
                        out_ap=out_d[:], in_ap=y_sb[:], idxs_ap=idx_reps[e][:],
                        num_idxs=CAPe, num_idxs_reg=creg2,
                        elem_size=H).then_inc(sc_sem, 16)
                    nc.gpsimd.free_register(creg2)
            with tc.tile_critical():
                nc.gpsimd.wait_ge(sc_sem, 16 * EL)

    nc.compile()
    return nc


_CACHE = {}


def _np_route(hidden, gate_w, e_bias):
    """f32 numpy clone of the device routing; returns dense cw [T, E]."""
    logits = (hidden @ gate_w.T).astype(np.float32)
    scores = (1.0 / (1.0 + np.exp(-logits))).astype(np.float32)
    swb = (scores + e_bias[None, :]).astype(np.float32)
    g = swb.reshape(T, N_GROUP, E // N_GROUP)
    gs = np.sort(g, axis=-1)[:, :, -2:].sum(-1, dtype=np.float32)
    thr_g = np.sort(gs, axis=-1)[:, -TOPK_GROUP:-TOPK_GROUP + 1]
    gmask = (gs >= thr_g).astype(np.float32)
    mswb = swb * np.repeat(gmask, E // N_GROUP, axis=-1)
    thr = np.sort(mswb, axis=-1)[:, -TOP_K:-TOP_K + 1]
    nmask = (mswb >= thr).astype(np.float32)
    s = scores * nmask
    s = s / (s.sum(-1, keepdims=True) + 1e-20) * ROUTED_SCALE
    return s


def _tok_wrap():
    """Token id for wrapped position: dst stream pos = p16*256 + f maps to
    src stream pos of the cw_loc[:, :, e] DMA: pos = p*TT + tt with
    p = pos // TT, tt = pos % TT; token = tt*128 + p."""
    pos = np.arange(T)
    tok = (pos % TT) * 128 + pos // TT
    return (tok.astype(np.float32) + 1.0).reshape(16, 1, T // 16)


def _plan(inputs):
    """Expert permutation (within-group sort by load) + per-slot caps."""
    hidden = np.asarray(inputs["hidden_states"], dtype=np.float32)
    gate_w = np.asarray(inputs["gate_w"], dtype=np.float32)
    e_bias = np.asarray(inputs["e_bias"], dtype=np.float32)
    cw = _np_route(hidden, gate_w, e_bias)
    counts = (cw > 0).sum(0)                      # [E]
    # within each group, order experts by descending load
    perm = np.zeros(E, dtype=np.int64)
    for gidx in range(N_GROUP):
        gsl = np.arange(gidx * EL, (gidx + 1) * EL)
        perm[gsl] = gsl[np.argsort(-counts[gsl], kind="stable")]
    pc = counts[perm].reshape(N_GROUP, EL)        # [group, slot]
    slot_max = pc.max(axis=0)                     # [EL]
    caps = tuple(int(-(-(c + 24) // 128) * 128) for c in slot_max)
    return perm, caps


def _host_prep(inputs, perm):
    hidden = np.ascontiguousarray(np.asarray(inputs["hidden_states"],
                                             dtype=np.float32))
    gate_w = np.asarray(inputs["gate_w"], dtype=np.float32)[perm]
    e_bias = np.asarray(inputs["e_bias"], dtype=np.float32)[perm]
    w1 = np.asarray(inputs["w1"], dtype=np.float32)[perm]
    w2 = np.asarray(inputs["w2"], dtype=np.float32)[perm]
    w3 = np.asarray(inputs["w3"], dtype=np.float32)[perm]
    ws1 = np.asarray(inputs["ws1"], dtype=np.float32)
    ws2 = np.asarray(inputs["ws2"], dtype=np.float32)
    ws3 = np.asarray(inputs["ws3"], dtype=np.float32)

    tok1 = np.broadcast_to(_tok_wrap(), (16, EL, T // 16)).copy()
    ident = np.eye(128, dtype=np.float32)
    nposc = -(np.arange(64)[None, :] * 128.0
              + np.arange(128)[:, None]).astype(np.float32)
    nposi = -(np.arange(512)[None, :] * 16.0
              + np.arange(16)[:, None]).astype(np.float32)

    in_maps = []
    for k in range(NCORES):
        es = slice(k * EL, (k + 1) * EL)
        isl = slice(k * ISL, (k + 1) * ISL)
        # move group k to the front; other groups keep order (whole groups)
        gperm = np.r_[np.arange(k * EL, (k + 1) * EL),
                      np.arange(0, k * EL), np.arange((k + 1) * EL, E)]
        in_maps.append({
            "hid": hidden,
            "gwt": np.ascontiguousarray(gate_w[gperm].T),
            "ebias": np.broadcast_to(e_bias[gperm], (128, E)).copy(),
            "tok1": tok1,
            "ident": ident,
            "nposc": nposc,
            "nposi": nposi,
            "w1t": np.ascontiguousarray(w1[es].transpose(0, 2, 1)),
            "w3t": np.ascontiguousarray(w3[es].transpose(0, 2, 1)),
            "w2t": np.ascontiguousarray(w2[es].transpose(0, 2, 1)),
            "ws1t": np.ascontiguousarray(ws1[isl].T),
            "ws3t": np.ascontiguousarray(ws3[isl].T),
            "ws2t": np.ascontiguousarray(ws2[:, isl].T),
        })
    return in_maps


def kernel(**inputs) -> np.ndarray:
    from concourse.bass_utils import run_bass_kernel_spmd

    perm, caps = _plan(inputs)
    if caps not in _CACHE:
        _CACHE[caps] = build_kernel(caps)
    nc = _CACHE[caps]
    in_maps = _host_prep(inputs, perm)
    res = run_bass_kernel_spmd(nc, in_maps, list(range(NCORES)))
    out = np.zeros((T, H), dtype=np.float32)
    for r in res.results:
        out += r["out"]
    return out



# revision 2
# speedup vs baseline: 6.6106x; 6.6106x over previous
"""DeepSeek-V3-style MoE kernel for Trainium2, 8-core expert-parallel.

Strategy (v2):
  - Routing runs on HOST in f32 (exactly mirrors the reference), producing
    dense combine weights cw [T, E]. The host performs the all-to-all token
    dispatch: for each core it gathers, pads and TRANSPOSES the selected
    token rows into xgt [H, CTOT] (bf16), so the device needs no on-device
    routing, no gather, and no PE transposes at all.
  - Experts are load-balanced: sort experts by token count (desc), slot j
    holds ranks [8j, 8j+8), one per core. All cores share one static cap
    per slot (SPMD requires identical shapes), caps are tight since ranks
    within an octile have similar counts.
  - Device = pure bf16 GEMM pipeline (fp32 PSUM accumulate):
      shared expert tensor-parallel over IS (2048 -> 256 per core) over all
      T tokens, then 8 routed expert slots. Weight-stationary matmuls:
      h1T/h3T [I, tok] = w @ xT, silu*mul on DVE/Act, down-proj back to
      [tok, H] with actT as stationary operand. Combine weight applied as a
      per-partition scalar on the PSUM->SBUF copy of y.
  - Outputs are bf16: shared partial [T, H] per core (host sums 8) and
    routed yout [CTOT, H] per core (host adds per-expert slices into the
    output; token lists within one expert are unique so vectorized
    fancy-index += is safe).
"""

import sys
import numpy as np

sys.path.insert(0, "/opt/trn_rl_repo")

T, H, E, I, IS = 4096, 1024, 64, 512, 2048
N_GROUP, TOPK_GROUP, TOP_K = 8, 4, 8
ROUTED_SCALE = 2.5

NCORES = 8
EL = E // NCORES          # expert slots per core
ISL = IS // NCORES        # shared intermediate slice per core
HB = H // 128             # 8
IB = I // 128             # 4
ISB = ISL // 128          # 2
NH = H // 512             # 2 (psum-bank halves of the down-proj)
TCH = 512                 # token chunk (psum bank limit, f32)


def build_kernel(caps):
    from concourse import bacc, mybir, tile

    f32 = mybir.dt.float32
    bf = mybir.dt.bfloat16
    AF = mybir.ActivationFunctionType
    OP = mybir.AluOpType

    CTOT = sum(caps)
    CBT = CTOT // 128

    nc = bacc.Bacc("TRN2", target_bir_lowering=False, debug=False,
                   num_devices=NCORES)

    hidT = nc.declare_dram_parameter("hidT", [H, T], bf, isOutput=False)
    xgt = nc.declare_dram_parameter("xgt", [H, CTOT], bf, isOutput=False)
    cwc = nc.declare_dram_parameter("cwc", [128, CBT], f32, isOutput=False)
    w13t = nc.declare_dram_parameter("w13t", [EL, H, 2 * I], bf,
                                     isOutput=False)
    w2t = nc.declare_dram_parameter("w2t", [EL, I, H], bf, isOutput=False)
    ws13t = nc.declare_dram_parameter("ws13t", [H, 2 * ISL], bf,
                                      isOutput=False)
    ws2t = nc.declare_dram_parameter("ws2t", [ISL, H], bf, isOutput=False)
    outs = nc.declare_dram_parameter("outs", [T, H], bf, isOutput=True)
    yout = nc.declare_dram_parameter("yout", [CTOT, H], bf, isOutput=True)

    import contextlib
    with tile.TileContext(nc) as tc, contextlib.ExitStack() as ctx:
        p_const = ctx.enter_context(tc.tile_pool(name="const", bufs=1))
        p_w = ctx.enter_context(tc.tile_pool(name="w", bufs=2))
        p_x = ctx.enter_context(tc.tile_pool(name="x", bufs=3))
        p_act = ctx.enter_context(tc.tile_pool(name="act", bufs=2))
        p_y = ctx.enter_context(tc.tile_pool(name="y", bufs=3))
        ps_h = ctx.enter_context(tc.tile_pool(name="ps_h", bufs=2,
                                              space="PSUM"))
        ps_y = ctx.enter_context(tc.tile_pool(name="ps_y", bufs=2,
                                              space="PSUM"))

        # shared-expert weights + combine weights, resident
        ws13_sb = p_const.tile([128, HB, 2 * ISL], bf, tag="ws13")
        for hb in range(HB):
            nc.sync.dma_start(out=ws13_sb[:, hb, :],
                              in_=ws13t[hb * 128:(hb + 1) * 128, :])
        ws2_sb = p_const.tile([128, ISB, H], bf, tag="ws2")
        for ib in range(ISB):
            nc.sync.dma_start(out=ws2_sb[:, ib, :],
                              in_=ws2t[ib * 128:(ib + 1) * 128, :])
        cw_sb = p_const.tile([128, CBT], f32, tag="cw")
        nc.sync.dma_start(out=cw_sb[:], in_=cwc[:])

        def gated_block(xT, wsb, nI, Nc, act_tag):
            """h1T/h3T -> silu*mul -> actT [128, nI-blocks, Nc] bf16."""
            actT = p_act.tile([128, nI, TCH], bf, tag=act_tag)
            for ib in range(nI):
                h1 = ps_h.tile([128, TCH], f32, tag="h1")
                h3 = ps_h.tile([128, TCH], f32, tag="h3")
                for hb in range(HB):
                    nc.tensor.matmul(
                        out=h1[:, :Nc],
                        lhsT=wsb[:, hb, ib * 128:(ib + 1) * 128],
                        rhs=xT[:, hb, :Nc],
                        start=(hb == 0), stop=(hb == HB - 1))
                for hb in range(HB):
                    nc.tensor.matmul(
                        out=h3[:, :Nc],
                        lhsT=wsb[:, hb, nI * 128 + ib * 128:
                                 nI * 128 + (ib + 1) * 128],
                        rhs=xT[:, hb, :Nc],
                        start=(hb == 0), stop=(hb == HB - 1))
                sil = p_act.tile([128, TCH], f32, tag="sil")
                nc.scalar.activation(sil[:, :Nc], h1[:, :Nc], AF.Silu)
                nc.vector.tensor_tensor(out=actT[:, ib, :Nc],
                                        in0=sil[:, :Nc], in1=h3[:, :Nc],
                                        op=OP.mult)
            return actT

        # ---------------- shared expert over all T tokens ----------------
        for c in range(T // TCH):
            hT = p_x.tile([128, HB, TCH], bf, tag="xT")
            for hb in range(HB):
                nc.sync.dma_start(
                    out=hT[:, hb, :],
                    in_=hidT[hb * 128:(hb + 1) * 128, c * TCH:(c + 1) * TCH])
            actT = gated_block(hT, ws13_sb, ISB, TCH, "actS")
            for tb in range(TCH // 128):
                y = ps_y.tile([128, H], f32, tag="y")
                for nh in range(NH):
                    for ib in range(ISB):
                        nc.tensor.matmul(
                            out=y[:, nh * 512:(nh + 1) * 512],
                            lhsT=actT[:, ib, tb * 128:(tb + 1) * 128],
                            rhs=ws2_sb[:, ib, nh * 512:(nh + 1) * 512],
                            start=(ib == 0), stop=(ib == ISB - 1))
                ysb = p_y.tile([128, H], bf, tag="ysb")
                nc.any.tensor_copy(out=ysb[:], in_=y[:])
                r0 = c * TCH + tb * 128
                nc.sync.dma_start(out=outs[r0:r0 + 128, :], in_=ysb[:])

        # ---------------- routed experts ----------------
        off = 0
        for j in range(EL):
            Cj = caps[j]
            if Cj == 0:
                continue
            w13sb = p_w.tile([128, HB, 2 * I], bf, tag="w13")
            for hb in range(HB):
                nc.sync.dma_start(out=w13sb[:, hb, :],
                                  in_=w13t[j, hb * 128:(hb + 1) * 128, :])
            w2sb = p_w.tile([128, IB, H], bf, tag="w2")
            for ib in range(IB):
                nc.sync.dma_start(out=w2sb[:, ib, :],
                                  in_=w2t[j, ib * 128:(ib + 1) * 128, :])
            for cc in range(0, Cj, TCH):
                Nc = min(TCH, Cj - cc)
                xT = p_x.tile([128, HB, TCH], bf, tag="xT")
                for hb in range(HB):
                    nc.sync.dma_start(
                        out=xT[:, hb, :Nc],
                        in_=xgt[hb * 128:(hb + 1) * 128,
                                off + cc:off + cc + Nc])
                actT = gated_block(xT, w13sb, IB, Nc, "actR")
                for tb in range(Nc // 128):
                    y = ps_y.tile([128, H], f32, tag="y")
                    for nh in range(NH):
                        for ib in range(IB):
                            nc.tensor.matmul(
                                out=y[:, nh * 512:(nh + 1) * 512],
                                lhsT=actT[:, ib, tb * 128:(tb + 1) * 128],
                                rhs=w2sb[:, ib, nh * 512:(nh + 1) * 512],
                                start=(ib == 0), stop=(ib == IB - 1))
                    ysb = p_y.tile([128, H], bf, tag="ysb")
                    gb = (off + cc) // 128 + tb
                    nc.vector.tensor_scalar_mul(ysb[:], y[:],
                                                cw_sb[:, gb:gb + 1])
                    r0 = off + cc + tb * 128
                    nc.sync.dma_start(out=yout[r0:r0 + 128, :], in_=ysb[:])
            off += Cj

    nc.compile()
    return nc


_CACHE = {}


def _np_route(hidden, gate_w, e_bias):
    """f32 numpy clone of the reference routing; returns dense cw [T, E]."""
    logits = (hidden @ gate_w.T).astype(np.float32)
    scores = (1.0 / (1.0 + np.exp(-logits))).astype(np.float32)
    swb = (scores + e_bias[None, :]).astype(np.float32)
    g = swb.reshape(T, N_GROUP, E // N_GROUP)
    gs = np.sort(g, axis=-1)[:, :, -2:].sum(-1, dtype=np.float32)
    thr_g = np.sort(gs, axis=-1)[:, -TOPK_GROUP:-TOPK_GROUP + 1]
    gmask = (gs >= thr_g).astype(np.float32)
    mswb = swb * np.repeat(gmask, E // N_GROUP, axis=-1)
    thr = np.sort(mswb, axis=-1)[:, -TOP_K:-TOP_K + 1]
    nmask = (mswb >= thr).astype(np.float32)
    s = scores * nmask
    s = s / (s.sum(-1, keepdims=True) + 1e-20) * ROUTED_SCALE
    return s


def _plan(inputs):
    """Routing + expert->(core, slot) assignment + static per-slot caps."""
    hidden = np.asarray(inputs["hidden_states"], dtype=np.float32)
    gate_w = np.asarray(inputs["gate_w"], dtype=np.float32)
    e_bias = np.asarray(inputs["e_bias"], dtype=np.float32)
    cw = _np_route(hidden, gate_w, e_bias)
    counts = (cw > 0).sum(0)                       # [E]
    order = np.argsort(-counts, kind="stable")
    assign = order.reshape(EL, NCORES)             # [slot, core] -> expert
    caps = tuple(int(-(-int(counts[assign[j]].max()) // 128) * 128)
                 for j in range(EL))
    return cw, assign, caps


def _host_prep(inputs, cw, assign, caps):
    import ml_dtypes
    bf16 = ml_dtypes.bfloat16

    hidden = np.asarray(inputs["hidden_states"], dtype=np.float32)
    w1 = np.asarray(inputs["w1"], dtype=np.float32)
    w2 = np.asarray(inputs["w2"], dtype=np.float32)
    w3 = np.asarray(inputs["w3"], dtype=np.float32)
    ws1 = np.asarray(inputs["ws1"], dtype=np.float32)
    ws2 = np.asarray(inputs["ws2"], dtype=np.float32)
    ws3 = np.asarray(inputs["ws3"], dtype=np.float32)

    CTOT = sum(caps)
    hidT = np.ascontiguousarray(hidden.T).astype(bf16)

    in_maps = []
    tok_lists = []
    for k in range(NCORES):
        isl = slice(k * ISL, (k + 1) * ISL)
        X = np.zeros((CTOT, H), dtype=np.float32)
        cwpad = np.zeros(CTOT, dtype=np.float32)
        toks_k = []
        offv = 0
        for j in range(EL):
            e = assign[j, k]
            tk = np.nonzero(cw[:, e] > 0)[0]
            n = len(tk)
            X[offv:offv + n] = hidden[tk]
            cwpad[offv:offv + n] = cw[tk, e]
            toks_k.append((offv, tk))
            offv += caps[j]
        es = assign[:, k]
        w13 = np.concatenate(
            [w1[es].transpose(0, 2, 1), w3[es].transpose(0, 2, 1)],
            axis=2)                                  # [EL, H, 2I]
        ws13 = np.concatenate([ws1[isl].T, ws3[isl].T], axis=1)  # [H, 2ISL]
        in_maps.append({
            "hidT": hidT,
            "xgt": np.ascontiguousarray(X.T).astype(bf16),
            "cwc": np.ascontiguousarray(
                cwpad.reshape(CTOT // 128, 128).T),
            "w13t": np.ascontiguousarray(w13).astype(bf16),
            "w2t": np.ascontiguousarray(w2[es].transpose(0, 2, 1)).astype(
                bf16),
            "ws13t": np.ascontiguousarray(ws13).astype(bf16),
            "ws2t": np.ascontiguousarray(ws2[:, isl].T).astype(bf16),
        })
        tok_lists.append(toks_k)
    return in_maps, tok_lists


def kernel(**inputs) -> np.ndarray:
    from concourse.bass_utils import run_bass_kernel_spmd

    cw, assign, caps = _plan(inputs)
    if caps not in _CACHE:
        _CACHE[caps] = build_kernel(caps)
    nc = _CACHE[caps]
    in_maps, tok_lists = _host_prep(inputs, cw, assign, caps)
    res = run_bass_kernel_spmd(nc, in_maps, list(range(NCORES)))
    out = np.zeros((T, H), dtype=np.float32)
    for k in range(NCORES):
        out += res.results[k]["outs"].astype(np.float32)
    for k in range(NCORES):
        yk = res.results[k]["yout"]
        for offv, tk in tok_lists[k]:
            if len(tk):
                out[tk] += yk[offv:offv + len(tk)].astype(np.float32)
    return out


# revision 3
# speedup vs baseline: 6.6641x; 1.0081x over previous
"""DeepSeek-V3-style MoE kernel for Trainium2, 8-core expert-parallel.

Strategy (v3):
  - Routing runs on HOST in f32 (exactly mirrors the reference), producing
    dense combine weights cw [T, E]. The host performs the all-to-all token
    dispatch: for each core it gathers, pads and TRANSPOSES the selected
    token rows into xgt [H, CTOT] (bf16), so the device needs no on-device
    routing, no gather, and no PE transposes at all.
  - Experts are load-balanced: sort experts by token count (desc), slot j
    holds ranks [8j, 8j+8), one per core. All cores share one static cap
    per slot (SPMD requires identical shapes); caps are 32-granular and
    tight since ranks within an octile have similar counts.
  - Device = pure bf16 GEMM pipeline (fp32 PSUM accumulate):
      shared expert tensor-parallel over IS (2048 -> 256 per core) over all
      T tokens, then 8 routed expert slots. Weight-stationary matmuls:
      h1T/h3T [I, tok] = w @ xT, silu*mul on Act/DVE, down-proj back to
      [tok, H] with actT as stationary operand. Combine weight applied as a
      per-partition scalar on the PSUM->SBUF copy of y. Expert weights are
      software-pipelined two slots ahead.
  - Outputs are bf16: shared partial [T, H] per core (host sums 8) and
    routed yout [CTOT, H] per core (host adds per-expert slices into the
    output; token lists within one expert are unique so vectorized
    fancy-index += is safe).
"""

import sys
import numpy as np

sys.path.insert(0, "/opt/trn_rl_repo")

T, H, E, I, IS = 4096, 1024, 64, 512, 2048
N_GROUP, TOPK_GROUP, TOP_K = 8, 4, 8
ROUTED_SCALE = 2.5

NCORES = 8
EL = E // NCORES          # expert slots per core
ISL = IS // NCORES        # shared intermediate slice per core
HB = H // 128             # 8
IB = I // 128             # 4
ISB = ISL // 128          # 2
NH = H // 512             # 2 (psum-bank halves of the down-proj)
TCH = 512                 # token chunk (psum bank limit, f32)
SCHUNKS = [256, 256] + [512] * 7   # shared-phase chunk plan (sum == T)


def build_kernel(caps):
    from concourse import bacc, mybir, tile

    f32 = mybir.dt.float32
    bf = mybir.dt.bfloat16
    AF = mybir.ActivationFunctionType
    OP = mybir.AluOpType

    CTOT = sum(caps)
    ncols = [-(-c // 128) for c in caps]          # cw columns per slot
    CBT = sum(ncols)

    nc = bacc.Bacc("TRN2", target_bir_lowering=False, debug=False,
                   num_devices=NCORES)

    hidT = nc.declare_dram_parameter("hidT", [H, T], bf, isOutput=False)
    xgt = nc.declare_dram_parameter("xgt", [H, CTOT], bf, isOutput=False)
    cwc = nc.declare_dram_parameter("cwc", [128, CBT], f32, isOutput=False)
    w13t = nc.declare_dram_parameter("w13t", [EL, H, 2 * I], bf,
                                     isOutput=False)
    w2t = nc.declare_dram_parameter("w2t", [EL, I, H], bf, isOutput=False)
    ws13t = nc.declare_dram_parameter("ws13t", [H, 2 * ISL], bf,
                                      isOutput=False)
    ws2t = nc.declare_dram_parameter("ws2t", [ISL, H], bf, isOutput=False)
    outs = nc.declare_dram_parameter("outs", [T, H], bf, isOutput=True)
    yout = nc.declare_dram_parameter("yout", [CTOT, H], bf, isOutput=True)

    import contextlib
    with tile.TileContext(nc) as tc, contextlib.ExitStack() as ctx:
        p_const = ctx.enter_context(tc.tile_pool(name="const", bufs=1))
        p_w = ctx.enter_context(tc.tile_pool(name="w", bufs=3))
        p_x = ctx.enter_context(tc.tile_pool(name="x", bufs=3))
        p_act = ctx.enter_context(tc.tile_pool(name="act", bufs=2))
        p_y = ctx.enter_context(tc.tile_pool(name="y", bufs=3))
        ps_h = ctx.enter_context(tc.tile_pool(name="ps_h", bufs=2,
                                              space="PSUM"))
        ps_y = ctx.enter_context(tc.tile_pool(name="ps_y", bufs=2,
                                              space="PSUM"))

        # shared-expert weights + combine weights, resident
        ws13_sb = p_const.tile([128, HB, 2 * ISL], bf, tag="ws13")
        nc.sync.dma_start(out=ws13_sb[:],
                          in_=ws13t[:].rearrange("(b p) i -> p b i", p=128))
        ws2_sb = p_const.tile([128, ISB, H], bf, tag="ws2")
        nc.sync.dma_start(out=ws2_sb[:],
                          in_=ws2t[:].rearrange("(b p) i -> p b i", p=128))
        cw_sb = p_const.tile([128, CBT], f32, tag="cw")
        nc.sync.dma_start(out=cw_sb[:], in_=cwc[:])

        def load_w(j):
            w13sb = p_w.tile([128, HB, 2 * I], bf, tag="w13")
            nc.sync.dma_start(
                out=w13sb[:],
                in_=w13t[j].rearrange("(b p) i -> p b i", p=128))
            w2sb = p_w.tile([128, IB, H], bf, tag="w2")
            nc.sync.dma_start(
                out=w2sb[:],
                in_=w2t[j].rearrange("(b p) i -> p b i", p=128))
            return w13sb, w2sb

        def gated_block(xT, wsb, nI, Nc, act_tag):
            """h1T/h3T -> silu*mul -> actT [128, nI-blocks, Nc] bf16."""
            actT = p_act.tile([128, nI, TCH], bf, tag=act_tag)
            for ib in range(nI):
                h1 = ps_h.tile([128, TCH], f32, tag="h1")
                h3 = ps_h.tile([128, TCH], f32, tag="h3")
                for hb in range(HB):
                    nc.tensor.matmul(
                        out=h1[:, :Nc],
                        lhsT=wsb[:, hb, ib * 128:(ib + 1) * 128],
                        rhs=xT[:, hb, :Nc],
                        start=(hb == 0), stop=(hb == HB - 1))
                for hb in range(HB):
                    nc.tensor.matmul(
                        out=h3[:, :Nc],
                        lhsT=wsb[:, hb, nI * 128 + ib * 128:
                                 nI * 128 + (ib + 1) * 128],
                        rhs=xT[:, hb, :Nc],
                        start=(hb == 0), stop=(hb == HB - 1))
                sil = p_act.tile([128, TCH], f32, tag="sil")
                nc.scalar.activation(sil[:, :Nc], h1[:, :Nc], AF.Silu)
                nc.vector.tensor_tensor(out=actT[:, ib, :Nc],
                                        in0=sil[:, :Nc], in1=h3[:, :Nc],
                                        op=OP.mult)
            return actT

        def down_proj(actT, wsb, nI, tb, r, cwap):
            """y [r, H] from actT cols [tb, tb+r); cwap None or [r,1]."""
            ysb = p_y.tile([128, H], bf, tag="ysb")
            for nh in range(NH):
                y = ps_y.tile([128, 512], f32, tag=f"y{nh}")
                for ib in range(nI):
                    nc.tensor.matmul(
                        out=y[:r, :],
                        lhsT=actT[:, ib, tb:tb + r],
                        rhs=wsb[:, ib, nh * 512:(nh + 1) * 512],
                        start=(ib == 0), stop=(ib == nI - 1))
                if cwap is None:
                    nc.any.tensor_copy(
                        out=ysb[:r, nh * 512:(nh + 1) * 512], in_=y[:r, :])
                else:
                    nc.vector.tensor_scalar_mul(
                        ysb[:r, nh * 512:(nh + 1) * 512], y[:r, :], cwap)
            return ysb

        # prefetch first two routed slots' weights
        wq = [load_w(0), load_w(1)]

        # ---------------- shared expert over all T tokens ----------------
        c0 = 0
        for Nc in SCHUNKS:
            hT = p_x.tile([128, HB, TCH], bf, tag="xT")
            nc.sync.dma_start(
                out=hT[:, :, :Nc],
                in_=hidT[:, c0:c0 + Nc].rearrange("(b p) t -> p b t", p=128))
            actT = gated_block(hT, ws13_sb, ISB, Nc, "actS")
            for tb in range(0, Nc, 128):
                ysb = down_proj(actT, ws2_sb, ISB, tb, 128, None)
                r0 = c0 + tb
                nc.sync.dma_start(out=outs[r0:r0 + 128, :], in_=ysb[:])
            c0 += Nc

        # ---------------- routed experts ----------------
        off = 0
        cwoff = 0
        for j in range(EL):
            Cj = caps[j]
            if Cj == 0:
                continue
            w13sb, w2sb = wq[j % 2]
            if j + 2 < EL and caps[j + 2] > 0:
                wq[j % 2] = load_w(j + 2)
            for cc in range(0, Cj, TCH):
                Nc = min(TCH, Cj - cc)
                xT = p_x.tile([128, HB, TCH], bf, tag="xT")
                nc.sync.dma_start(
                    out=xT[:, :, :Nc],
                    in_=xgt[:, off + cc:off + cc + Nc].rearrange(
                        "(b p) t -> p b t", p=128))
                actT = gated_block(xT, w13sb, IB, Nc, "actR")
                for tb in range(0, Nc, 128):
                    r = min(128, Nc - tb)
                    gb = cwoff + (cc + tb) // 128
                    ysb = down_proj(actT, w2sb, IB, tb, r,
                                    cw_sb[:r, gb:gb + 1])
                    r0 = off + cc + tb
                    nc.sync.dma_start(out=yout[r0:r0 + r, :], in_=ysb[:r, :])
            off += Cj
            cwoff += -(-Cj // 128)

    nc.compile()
    return nc


_CACHE = {}


def _bf16(x):
    """Fast f32 -> bf16 (round to nearest even) via bit manipulation."""
    import ml_dtypes
    v = np.ascontiguousarray(x, dtype=np.float32).view(np.uint32)
    r = ((v + 0x7FFF + ((v >> 16) & 1)) >> 16).astype(np.uint16)
    return r.view(ml_dtypes.bfloat16)


def _np_route(hidden, gate_w, e_bias):
    """f32 numpy clone of the reference routing; returns dense cw [T, E]."""
    logits = (hidden @ gate_w.T).astype(np.float32)
    scores = (1.0 / (1.0 + np.exp(-logits))).astype(np.float32)
    swb = (scores + e_bias[None, :]).astype(np.float32)
    g = swb.reshape(T, N_GROUP, E // N_GROUP)
    gs = np.sort(g, axis=-1)[:, :, -2:].sum(-1, dtype=np.float32)
    thr_g = np.sort(gs, axis=-1)[:, -TOPK_GROUP:-TOPK_GROUP + 1]
    gmask = (gs >= thr_g).astype(np.float32)
    mswb = swb * np.repeat(gmask, E // N_GROUP, axis=-1)
    thr = np.sort(mswb, axis=-1)[:, -TOP_K:-TOP_K + 1]
    nmask = (mswb >= thr).astype(np.float32)
    s = scores * nmask
    s = s / (s.sum(-1, keepdims=True) + 1e-20) * ROUTED_SCALE
    return s


def _plan(inputs):
    """Routing + expert->(core, slot) assignment + static per-slot caps."""
    hidden = np.asarray(inputs["hidden_states"], dtype=np.float32)
    gate_w = np.asarray(inputs["gate_w"], dtype=np.float32)
    e_bias = np.asarray(inputs["e_bias"], dtype=np.float32)
    cw = _np_route(hidden, gate_w, e_bias)
    counts = (cw > 0).sum(0)                       # [E]
    order = np.argsort(-counts, kind="stable")
    assign = order.reshape(EL, NCORES)             # [slot, core] -> expert
    caps = tuple(int(-(-int(counts[assign[j]].max()) // 32) * 32)
                 for j in range(EL))
    return cw, assign, caps


def _host_prep(inputs, cw, assign, caps):
    hidden = np.asarray(inputs["hidden_states"], dtype=np.float32)
    w1 = np.asarray(inputs["w1"], dtype=np.float32)
    w2 = np.asarray(inputs["w2"], dtype=np.float32)
    w3 = np.asarray(inputs["w3"], dtype=np.float32)
    ws1 = np.asarray(inputs["ws1"], dtype=np.float32)
    ws2 = np.asarray(inputs["ws2"], dtype=np.float32)
    ws3 = np.asarray(inputs["ws3"], dtype=np.float32)

    CTOT = sum(caps)
    ncols = [-(-c // 128) for c in caps]
    CBT = sum(ncols)
    hidT = _bf16(hidden.T)

    # global weight prep (transposed, bf16), sliced per core afterwards
    w13_all = _bf16(np.concatenate(
        [w1.transpose(0, 2, 1), w3.transpose(0, 2, 1)], axis=2))  # [E,H,2I]
    w2t_all = _bf16(w2.transpose(0, 2, 1))                        # [E,I,H]

    in_maps = []
    tok_lists = []
    for k in range(NCORES):
        isl = slice(k * ISL, (k + 1) * ISL)
        X = np.zeros((CTOT, H), dtype=np.float32)
        cwpad = np.zeros(CBT * 128, dtype=np.float32)
        toks_k = []
        offv = 0
        offc = 0
        for j in range(EL):
            e = assign[j, k]
            tk = np.nonzero(cw[:, e] > 0)[0]
            n = len(tk)
            X[offv:offv + n] = hidden[tk]
            cwpad[offc:offc + n] = cw[tk, e]
            toks_k.append((offv, tk))
            offv += caps[j]
            offc += ncols[j] * 128
        es = assign[:, k]
        ws13 = np.concatenate([ws1[isl].T, ws3[isl].T], axis=1)  # [H, 2ISL]
        in_maps.append({
            "hidT": hidT,
            "xgt": _bf16(X.T),
            "cwc": np.ascontiguousarray(cwpad.reshape(CBT, 128).T),
            "w13t": np.ascontiguousarray(w13_all[es]),
            "w2t": np.ascontiguousarray(w2t_all[es]),
            "ws13t": _bf16(ws13),
            "ws2t": _bf16(ws2[:, isl].T),
        })
        tok_lists.append(toks_k)
    return in_maps, tok_lists


def kernel(**inputs) -> np.ndarray:
    from concourse.bass_utils import run_bass_kernel_spmd

    cw, assign, caps = _plan(inputs)
    if caps not in _CACHE:
        _CACHE[caps] = build_kernel(caps)
    nc = _CACHE[caps]
    in_maps, tok_lists = _host_prep(inputs, cw, assign, caps)
    res = run_bass_kernel_spmd(nc, in_maps, list(range(NCORES)))
    out = np.zeros((T, H), dtype=np.float32)
    for k in range(NCORES):
        out += res.results[k]["outs"].astype(np.float32)
    for k in range(NCORES):
        yk = res.results[k]["yout"]
        for offv, tk in tok_lists[k]:
            if len(tk):
                out[tk] += yk[offv:offv + len(tk)].astype(np.float32)
    return out
